# revision 9
# baseline (speedup 1.0000x reference)
"""AdditiveAttention pooling kernel for 8 trn2 NeuronCores.

out = softmax(v @ tanh(h @ W.T + b).T) @ h   for h [131072, 768].

Strategy: shard rows of h across 8 cores; one pass per shard. The z = W@h
matmul runs in fp8 (DoubleRow perf mode: 256-deep contraction per
instruction, 2x bf16 throughput on TRN2). fp8 alone pushes rel-err past
the 2e-2 gate, so the scores carry a *linear-split correction*: with
g = v16.W exact and g~8 = v16.W8/16 the exact linear response of the
quantized matmul (both host-side, f64),

    s = v.tanh(z~+b) + [g.h - g~8.h8]

The tanh nonlinearity damps the fp8 matmul error (x tanh^2 ~ 0.3) and the
bracketed linear term restores the exact rank-1 part. That term is two
host matvecs (0.2% of the kernel FLOPs) shipped as a tiny per-row f16
tensor l = 256*(g.h - g~8.h8), folded into the score PSUM by one
identity-stationary 4-column matmul. All score terms sit at a x256 scale
(v' = 256 v), un-scaled for free by the exp's scale=1/256. Measured
rel-err vs f32 reference: 0.0082.

Per 512-row block, software-pipelined (ws two blocks deep so the alpha
DRAM-bounce round trip never stalls DVE):
  PE : z~ = 16W8 @ h8 per jc (3 DoubleRow matmuls, 256-contraction each);
       scores = identity@l + 24 u-stationary bf16 1-col matmuls into
       s_col [128,4]; PE transpose -> [4,128]
  ACT: tanh(z~/16 + b) per-partition bias; exp(s/256 - 10) with accum_out
       producing the per-block softmax denominator for free
  DMA: alpha row [1,512] bounced through DRAM and read back with a
       stride-0 broadcast AP -> ab [128,512]
  DVE/POOL: fused weighted sum on the fp8 h tiles via
       scalar_tensor_tensor, split 3 chunks DVE + 3 chunks GpSimd:
       P[m, blk] = sum_i ab[m,i] * h8[m,i]
Outputs: part [128, 6] f32 and den [4, 32] f32; host divides num/den.
"""

import numpy as np
import ml_dtypes

import concourse.bass as bass
import concourse.tile as tile
from concourse import mybir
from concourse.bass_utils import run_bass_kernel_spmd
from concourse.vector_clock import ScopedClock, VectorClock

N = 131072
H = 768
NCORES = 8
SHARD = N // NCORES          # 16384 rows per core
R = 512                      # rows per block
NB = SHARD // R              # 32 blocks
MC = H // 128                # 6 chunks of the feature dim
NC2 = MC // 2                # 3 DoubleRow pair-chunks (256 features each)
EXP_SHIFT = -10.0            # exp(s - 10): keeps a_i comfortably in bf16
SW = 16.0                    # W pre-scale into fp8
SS = 256.0                   # score scale (v' = SS*v); exp applies 1/SS

BF16 = mybir.dt.bfloat16
F16 = mybir.dt.float16
F32 = mybir.dt.float32
FP8 = mybir.dt.float8e4
DR = mybir.MatmulPerfMode.DoubleRow
NPBF16 = ml_dtypes.bfloat16
NPFP8 = ml_dtypes.float8_e4m3

_ENGINE_CACHE = {}


def _patch_tail_drain():
    """This walrus build rejects instructions carrying >1 semaphore wait.

    1) Tile's end-of-context drain aggregates one wait per live processor
       onto a single SP Drain — split into one SP nop per pending processor
       tick, then a wait-free drain (same engine + program order: equivalent).
    2) Tile's wait assignment can attach 2-3 waits to body instructions.
       Before lowering, hoist all but one wait of each instruction onto
       same-engine NOPs inserted right before it (engine queues are FIFO)."""
    if getattr(tile.TileContext, "_ant_tail_patch", False):
        return

    def _drain_and_barrier(self, tick_clock, wait_clock):
        gvc = tick_clock.global_clock
        n = len(gvc)
        for p in range(n):
            t = gvc[p]
            if t > 0:
                req = [0] * n
                req[p] = t
                nop = self.nc.sync.nop()
                wait_clock.add_sem_waits(
                    nop.ins, ScopedClock({None: VectorClock(req)})
                )
        self.nc.sync.drain()
        self.nc.all_engine_barrier()
        popped = self.nc._tile_sem_poison_stack.pop()
        assert popped is self._sem_poison
        self.nc.clear_and_free_semaphores(list(self.sems.allocated().values()))
        self.nc.all_engine_barrier()

    tile.TileContext._drain_and_barrier = _drain_and_barrier

    orig_lower = tile.TileContext._lower_ordered_insts

    def _lower_with_wait_split(self, ordered):
        for insts in ordered.values():
            out = []
            for inst in insts:
                si = getattr(inst, "sync_info", None)
                if si is not None and len(si.on_wait) > 1:
                    waits = list(si.on_wait)
                    eng = inst.engine
                    for w in waits[:-1]:
                        nop = mybir.InstNoOp(
                            name=self.nc.get_next_instruction_name(),
                            ins=[],
                            outs=[],
                            engine=eng,
                        )
                        nop.sync_info = mybir.SyncInfo(on_wait=[w], on_update=[])
                        out.append(nop)
                    inst.sync_info = mybir.SyncInfo(
                        on_wait=[waits[-1]], on_update=list(si.on_update)
                    )
                out.append(inst)
            insts[:] = out
        return orig_lower(self, ordered)

    tile.TileContext._lower_ordered_insts = _lower_with_wait_split
    tile.TileContext._ant_tail_patch = True


def build_kernel():
    _patch_tail_drain()
    nc = bass.Bass("TRN2", debug=False)

    # block-major fp8 h: [NB, 128, 3*2*512] contiguous per block
    h8d = nc.dram_tensor("h8", [NB, 128, NC2 * 2 * R], FP8, kind="ExternalInput").ap()
    w8d = nc.dram_tensor("w8", [128, MC * NC2 * 2 * 128], FP8, kind="ExternalInput").ap()
    lcd = nc.dram_tensor("lcorr", [NB, 128, 4], F16, kind="ExternalInput").ap()
    bT = nc.dram_tensor("bT", [128, MC], F32, kind="ExternalInput").ap()
    vS = nc.dram_tensor("vS", [128, MC], BF16, kind="ExternalInput").ap()
    ident = nc.dram_tensor("ident", [128, 128], F16, kind="ExternalInput").ap()
    bounce = nc.dram_tensor("bounce", [NB, R], BF16, kind="Internal").ap()
    part_out = nc.dram_tensor("part", [128, MC], F32, kind="ExternalOutput").ap()
    den_out = nc.dram_tensor("den", [4, NB], F32, kind="ExternalOutput").ap()

    with tile.TileContext(nc) as tc:
        with tc.tile_pool(name="consts", bufs=1) as consts, \
             tc.tile_pool(name="h8p", bufs=5) as h8_pool, \
             tc.tile_pool(name="h0p", bufs=1) as h0_pool, \
             tc.tile_pool(name="lcp", bufs=3) as lc_pool, \
             tc.tile_pool(name="uTp", bufs=8) as uT_pool, \
             tc.tile_pool(name="arow", bufs=2) as arow_pool, \
             tc.tile_pool(name="scp", bufs=2) as sc_pool, \
             tc.tile_pool(name="abp", bufs=3) as ab_pool, \
             tc.tile_pool(name="ttout", bufs=4) as tt_pool, \
             tc.tile_pool(name="zps", bufs=5, space="PSUM") as z_pool, \
             tc.tile_pool(name="sps", bufs=1, space="PSUM") as s_pool, \
             tc.tile_pool(name="a4ps", bufs=1, space="PSUM") as a4_pool, \
             tc.tile_pool(name="wps", bufs=1, space="PSUM") as warm_pool, \
             tc.tile_pool(name="outp", bufs=1) as out_pool:

            # --- constants loaded once (scalar-engine trigger: its own
            # DMA queue, so nothing queues behind block 0's h8). Split W8
            # in halves so block 0's first z-group waits on less. ---
            w8_sb = consts.tile([128, MC, NC2, 2, 128], FP8)
            half = MC // 2 * NC2 * 2 * 128
            nc.scalar.dma_start(
                w8_sb[:].rearrange("p jc c j m -> p (jc c j m)")[:, 0:half],
                w8d[:, 0:half],
            )
            nc.scalar.dma_start(
                w8_sb[:].rearrange("p jc c j m -> p (jc c j m)")[:, half:2 * half],
                w8d[:, half:2 * half],
            )
            bT_sb = consts.tile([128, MC], F32)
            nc.scalar.dma_start(bT_sb[:], bT)
            vS_sb = consts.tile([128, MC], BF16)
            nc.scalar.dma_start(vS_sb[:], vS)
            id_sb = consts.tile([128, 128], F16)
            nc.scalar.dma_start(id_sb[:], ident)
            shift_sb = consts.tile([4, 1], F32)
            nc.vector.memset(shift_sb[:], EXP_SHIFT)

            den_sb = out_pool.tile([4, NB], F32)
            pcol_sb = out_pool.tile([128, MC * NB], F32)  # [p, mc*NB + blk]
            part_sb = out_pool.tile([128, MC], F32)

            # Warm-up while the first DMAs are in flight: ~8.5us of dummy
            # matmuls flips the PE HAM clock gate to 2.4 GHz before real work
            # arrives, and a tiny tanh+exp pulls the ACT table loads off the
            # critical path. All operate on a zeroed scratch tile.
            warm_sb = consts.tile([128, 512], BF16)
            nc.vector.memset(warm_sb[:], 0.0)
            warm_ps = warm_pool.tile([128, R], F32, tag="warm")
            NWARM = 20
            for wi in range(NWARM):
                nc.tensor.matmul(
                    warm_ps[:],
                    lhsT=warm_sb[:, 0:128],
                    rhs=warm_sb[:],
                    start=(wi == 0),
                    stop=(wi == NWARM - 1),
                    skip_group_check=True,
                )
            warm_u = uT_pool.tile([128, 32], BF16, tag="warm_u")
            nc.scalar.activation(
                warm_u[:], warm_ps[:, 0:32],
                mybir.ActivationFunctionType.Tanh, bias=0.0, scale=1.0,
            )
            nc.scalar.activation(
                warm_u[:], warm_ps[:, 0:32],
                mybir.ActivationFunctionType.Exp, bias=0.0, scale=1.0,
            )

            # per-block live state for the software pipeline
            state = {}

            def h8_rhs(st, c):
                """[128, 2, 512] DoubleRow moving slice for pair-chunk c."""
                if "h8_chunks" in st:
                    return st["h8_chunks"][c][:]
                return st["h8"][:, c, :, :]

            def h8_ws(st, c, j):
                """[128, 512] fp8 chunk for the weighted sum (mc = 2c+j)."""
                if "h8_chunks" in st:
                    return st["h8_chunks"][c][:, j, :]
                return st["h8"][:, c, j, :]

            def emit_s_batch(blk):
                """Scores for block blk: one identity-stationary 4-col matmul
                injects the host-computed linear correction l, then per ic
                column 6 u-stationary bf16 1-col matmuls (v' = 256v), one
                sequential PSUM accumulation region."""
                st = state[blk]
                s_col = s_pool.tile([128, 4], F32)
                nc.tensor.matmul(
                    s_col[:],
                    lhsT=id_sb[:],
                    rhs=st["lc"][:],
                    start=True,
                    stop=False,
                    skip_group_check=True,
                )
                # ic outer: accumulation groups must be sequential — this
                # hardware corrupts interleaved groups within one PSUM bank
                for ic in range(4):
                    for jc in range(MC):
                        nc.tensor.matmul(
                            s_col[:, ic : ic + 1],
                            lhsT=st["u"][jc][:, ic * 128 : (ic + 1) * 128],
                            rhs=vS_sb[:, jc : jc + 1],
                            start=False,
                            stop=(jc == MC - 1),
                            skip_group_check=True,
                        )
                st.pop("lc", None)
                sc16 = sc_pool.tile([128, 4], F16)
                nc.vector.tensor_copy(sc16[:], s_col[:])
                st["sc16"] = sc16

            def emit_transpose(blk):
                st = state[blk]
                a4_ps = a4_pool.tile([4, 128], F16)
                nc.tensor.transpose(a4_ps[:], st["sc16"][:], id_sb[:])
                st["a4_ps"] = a4_ps

            def emit_exp_bounce(blk):
                """exp(s/256 - 10) + accum denominators, then DRAM bounce."""
                st = state[blk]
                a4row = arow_pool.tile([4, 128], BF16)
                nc.scalar.activation(
                    a4row[:], st["a4_ps"][:], mybir.ActivationFunctionType.Exp,
                    bias=shift_sb[:], scale=1.0 / SS,
                )
                # den on DVE: keeps ACT off the ACCUM flush path
                nc.vector.tensor_reduce(
                    den_sb[:, blk : blk + 1], a4row[:],
                    axis=mybir.AxisListType.X, op=mybir.AluOpType.add,
                )
                nc.sync.dma_start(
                    bounce[blk : blk + 1, :].rearrange("b (c q) -> (b c) q", c=4),
                    a4row[:],
                )
                ab = ab_pool.tile([128, R], BF16)
                src = bass.AP(
                    tensor=bounce.tensor,
                    offset=bounce.offset + blk * R,
                    ap=[[0, 128], [128, 4], [1, 128]],
                )
                nc.sync.dma_start(
                    ab[:].rearrange("p (c q) -> p c q", c=4), src
                )
                st["ab"] = ab

            def emit_ws(blk):
                """fused weighted sum on fp8 h: pcol[:, mc*NB+blk] += ab*h8.
                (GpSimd can't run TensorScalarPtr — walrus ISA check — so
                all six chunks go to DVE.)"""
                st = state[blk]
                for c in range(NC2):
                    for j in range(2):
                        mc = 2 * c + j
                        eng = nc.vector
                        tt = tt_pool.tile([128, R], BF16)
                        eng.scalar_tensor_tensor(
                            out=tt[:],
                            in0=h8_ws(st, c, j),
                            scalar=1.0,
                            in1=st["ab"][:],
                            op0=mybir.AluOpType.mult,
                            op1=mybir.AluOpType.mult,
                            accum_out=pcol_sb[:, mc * NB + blk : mc * NB + blk + 1],
                        )
                state[blk] = {}

            for blk in range(NB):
                if blk == 0:
                    # block 0: one DMA per pair-chunk, alternating trigger
                    # engines, so the first z-group starts earlier
                    chunks = []
                    for c in range(NC2):
                        t = h0_pool.tile([128, 2, R], FP8, tag=f"h0{c}")
                        eng = nc.gpsimd if c % 2 == 0 else nc.sync
                        eng.dma_start(
                            t[:].rearrange("p j i -> p (j i)"),
                            h8d[0, :, c * 2 * R : (c + 1) * 2 * R],
                        )
                        chunks.append(t)
                    state[blk] = {"h8_chunks": chunks, "u": {}}
                else:
                    h8_t = h8_pool.tile([128, NC2, 2, R], FP8)
                    nc.gpsimd.dma_start(
                        h8_t[:].rearrange("p c j i -> p (c j i)"), h8d[blk]
                    )
                    state[blk] = {"h8": h8_t, "u": {}}
                lc_t = lc_pool.tile([128, 4], F16)
                nc.sync.dma_start(lc_t[:], lcd[blk])
                state[blk]["lc"] = lc_t

                for jc in range(MC):
                    z_ps = z_pool.tile([128, R], F32)
                    for c in range(NC2):
                        nc.tensor.matmul(
                            z_ps[:],
                            lhsT=w8_sb[:, jc, c, :, :],
                            rhs=h8_rhs(state[blk], c),
                            start=(c == 0),
                            stop=(c == NC2 - 1),
                            perf_mode=DR,
                        )
                    uT_t = uT_pool.tile([128, R], BF16)
                    nc.scalar.activation(
                        uT_t[:], z_ps[:], mybir.ActivationFunctionType.Tanh,
                        bias=bT_sb[:, jc : jc + 1], scale=1.0 / SW,
                    )
                    state[blk]["u"][jc] = uT_t

                    # pipelined tail work for earlier blocks, slotted between
                    # z-groups so no engine stalls on another. ws runs two
                    # blocks deep: the alpha bounce round trip gets a full
                    # block of slack.
                    if jc == 1 and blk >= 2:
                        emit_ws(blk - 2)
                    elif jc == 2 and blk >= 1:
                        emit_s_batch(blk - 1)
                    elif jc == 3 and blk >= 1:
                        emit_transpose(blk - 1)
                    elif jc == 4 and blk >= 1:
                        emit_exp_bounce(blk - 1)

            # drain the pipeline
            emit_s_batch(NB - 1)
            emit_transpose(NB - 1)
            emit_exp_bounce(NB - 1)
            emit_ws(NB - 2)
            emit_ws(NB - 1)

            # fold block partials: part[:, mc] = sum_blk pcol[:, mc*NB+blk]
            for mc in range(MC):
                nc.vector.tensor_reduce(
                    part_sb[:, mc : mc + 1],
                    pcol_sb[:, mc * NB : (mc + 1) * NB],
                    axis=mybir.AxisListType.X,
                    op=mybir.AluOpType.add,
                )

            nc.gpsimd.dma_start(part_out, part_sb[:])
            nc.gpsimd.dma_start(den_out, den_sb[:])

    return nc


def _get_engine():
    if "nc" not in _ENGINE_CACHE:
        _ENGINE_CACHE["nc"] = build_kernel()
    return _ENGINE_CACHE["nc"]


def _feat_major_to_blocks(x_fp8):
    """[H, SHARD] fp8 feature-major -> [NB, 128, 3*2*512] block-major."""
    a = np.asarray(x_fp8).reshape(NC2, 2, 128, NB, R)       # c j p blk i
    return np.ascontiguousarray(a.transpose(3, 2, 0, 1, 4).reshape(
        NB, 128, NC2 * 2 * R))


def make_in_maps(inputs):
    h_i = np.asarray(inputs["h_i"], dtype=np.float32)
    W_weight = np.asarray(inputs["W_weight"], dtype=np.float32)
    W_bias = np.asarray(inputs["W_bias"], dtype=np.float32)
    v = np.asarray(inputs["v"], dtype=np.float32)

    # fp8 h (feature-major)
    hT = np.ascontiguousarray(h_i.T)                # [H, N]
    h8 = hT.astype(NPFP8)

    # W8 stationary tiles [p, jc, c, j, m] = 16*W[jc*128+m, (2c+j)*128+p]
    W8 = (W_weight * SW).astype(NPFP8)
    W8dr = np.ascontiguousarray(
        W8.reshape(MC, 128, NC2, 2, 128).transpose(4, 0, 2, 3, 1)
        .reshape(128, MC * NC2 * 2 * 128))

    # exact linear-split correction, two host matvecs (f32 BLAS):
    #   l_i = SS * (g.h_i - g~8.h8_i)
    v16 = v.astype(NPBF16).astype(np.float32).ravel()
    g = (v16.astype(np.float64) @ W_weight.astype(np.float64)).astype(np.float32)
    g8t = ((v16.astype(np.float64) @ (W8.astype(np.float64))) / SW).astype(np.float32)
    l_full = SS * (g @ hT - g8t @ h8.astype(np.float32))     # [N] f32
    l16 = l_full.astype(np.float16)

    bT = np.ascontiguousarray(W_bias.reshape(MC, 128).T)
    vSa = np.ascontiguousarray(
        (v16 * SS).astype(NPBF16).reshape(MC, 128).T)
    identm = np.eye(128, dtype=np.float16)

    in_maps = []
    for c in range(NCORES):
        r0, r1 = c * SHARD, (c + 1) * SHARD
        lcorr = np.ascontiguousarray(
            l16[r0:r1].reshape(NB, 4, 128).transpose(0, 2, 1))
        in_maps.append({
            "h8": _feat_major_to_blocks(h8[:, r0:r1]),
            "w8": W8dr,
            "lcorr": lcorr,
            "bT": bT,
            "vS": vSa,
            "ident": identm,
        })
    return in_maps


def kernel(h_i, W_weight, W_bias, v, trace=False):
    in_maps = make_in_maps(
        {"h_i": h_i, "W_weight": W_weight, "W_bias": W_bias, "v": v}
    )
    nc = _get_engine()
    res = run_bass_kernel_spmd(
        nc, in_maps, core_ids=list(range(NCORES)), trace=trace
    )
    _ENGINE_CACHE["last_results"] = res

    num = np.zeros(H, dtype=np.float64)
    den = 0.0
    for c in range(NCORES):
        # part [128, MC]: element [p, mc] is the shard partial for
        # feature m = mc*128 + p
        part = res.results[c]["part"].astype(np.float64)
        num += part.T.reshape(H)
        den += res.results[c]["den"].astype(np.float64).sum()
    out = (num / den).astype(np.float32).reshape(1, H)
    return out


if __name__ == "__main__":
    rng = np.random.default_rng(0)
    h = rng.standard_normal((N, H), dtype=np.float32)
    W = (rng.standard_normal((H, H)) * 0.02).astype(np.float32)
    b = (rng.standard_normal(H) * 0.02).astype(np.float32)
    vv = (rng.standard_normal((1, H)) * 0.1).astype(np.float32)
    out = kernel(h, W, b, vv)
    u = np.tanh(h.astype(np.float64) @ W.astype(np.float64).T + b)
    s = (vv.astype(np.float64) @ u.T).ravel()
    a = np.exp(s - s.max())
    ref = (a @ h.astype(np.float64)) / a.sum()
    rel = np.linalg.norm(out.ravel() - ref) / np.linalg.norm(ref)
    print("rel err vs fp64 numpy ref:", rel)


# revision 12
# speedup vs baseline: 1.3631x; 1.3631x over previous
"""AdditiveAttention pooling kernel for 8 trn2 NeuronCores.

out = softmax(v @ tanh(h @ W.T + b).T) @ h   for h [131072, 768].

Strategy: shard rows of h across 8 cores; one pass per shard. The z = W@h
matmul runs in fp8 (DoubleRow perf mode: 256-deep contraction per
instruction, 2x bf16 throughput on TRN2). fp8 alone pushes rel-err past
the 2e-2 gate, so the scores carry a *linear-split correction*: with
g = v16.W exact and g~8 = v16.W8/16 the exact linear response of the
quantized matmul (both host-side, f64),

    s = v.tanh(z~+b) + [g.h - g~8.h8]

The tanh nonlinearity damps the fp8 matmul error (x tanh^2 ~ 0.3) and the
bracketed linear term restores the exact rank-1 part. That term is two
host matvecs (0.2% of the kernel FLOPs) shipped as a tiny per-row f16
tensor l = 256*(g.h - g~8.h8), folded into the score PSUM by one
identity-stationary 4-column matmul. All score terms sit at a x256 scale
(v' = 256 v), un-scaled for free by the exp's scale=1/256. Measured
rel-err vs f32 reference: 0.0082.

Per 512-row block, software-pipelined (ws two blocks deep so the alpha
DRAM-bounce round trip never stalls DVE):
  PE : z~ = 16W8 @ h8 per jc (3 DoubleRow matmuls, 256-contraction each);
       scores = identity@l + 24 u-stationary bf16 1-col matmuls into
       s_col [128,4]; PE transpose -> [4,128]
  ACT: tanh(z~/16 + b) per-partition bias; exp(s/256 - 10) with accum_out
       producing the per-block softmax denominator for free
  DMA: alpha row [1,512] bounced through DRAM and read back with a
       stride-0 broadcast AP -> ab [128,512]
  DVE/POOL: fused weighted sum on the fp8 h tiles via
       scalar_tensor_tensor, split 3 chunks DVE + 3 chunks GpSimd:
       P[m, blk] = sum_i ab[m,i] * h8[m,i]
Outputs: part [128, 6] f32 and den [4, 32] f32; host divides num/den.
"""

import numpy as np
import ml_dtypes

import concourse.bass as bass
import concourse.tile as tile
from concourse import mybir
from concourse.bass_utils import run_bass_kernel_spmd
from concourse.vector_clock import ScopedClock, VectorClock

N = 131072
H = 768
NCORES = 8
SHARD = N // NCORES          # 16384 rows per core
R = 512                      # rows per block
NB = SHARD // R              # 32 blocks
MC = H // 128                # 6 chunks of the feature dim
NC2 = MC // 2                # 3 DoubleRow pair-chunks (256 features each)
EXP_SHIFT = -10.0            # exp(s - 10): keeps a_i comfortably in bf16
SW = 16.0                    # W pre-scale into fp8
SS = 256.0                   # score scale (v' = SS*v); exp applies 1/SS

BF16 = mybir.dt.bfloat16
F16 = mybir.dt.float16
F32 = mybir.dt.float32
FP8 = mybir.dt.float8e4
DR = mybir.MatmulPerfMode.DoubleRow
NPBF16 = ml_dtypes.bfloat16
NPFP8 = ml_dtypes.float8_e4m3

_ENGINE_CACHE = {}


def _patch_tail_drain():
    """This walrus build rejects instructions carrying >1 semaphore wait.

    1) Tile's end-of-context drain aggregates one wait per live processor
       onto a single SP Drain — split into one SP nop per pending processor
       tick, then a wait-free drain (same engine + program order: equivalent).
    2) Tile's wait assignment can attach 2-3 waits to body instructions.
       Before lowering, hoist all but one wait of each instruction onto
       same-engine NOPs inserted right before it (engine queues are FIFO)."""
    if getattr(tile.TileContext, "_ant_tail_patch", False):
        return

    def _drain_and_barrier(self, tick_clock, wait_clock):
        gvc = tick_clock.global_clock
        n = len(gvc)
        for p in range(n):
            t = gvc[p]
            if t > 0:
                req = [0] * n
                req[p] = t
                nop = self.nc.sync.nop()
                wait_clock.add_sem_waits(
                    nop.ins, ScopedClock({None: VectorClock(req)})
                )
        self.nc.sync.drain()
        self.nc.all_engine_barrier()
        popped = self.nc._tile_sem_poison_stack.pop()
        assert popped is self._sem_poison
        self.nc.clear_and_free_semaphores(list(self.sems.allocated().values()))
        self.nc.all_engine_barrier()

    tile.TileContext._drain_and_barrier = _drain_and_barrier

    orig_lower = tile.TileContext._lower_ordered_insts

    def _lower_with_wait_split(self, ordered):
        for insts in ordered.values():
            out = []
            for inst in insts:
                si = getattr(inst, "sync_info", None)
                if si is not None and len(si.on_wait) > 1:
                    waits = list(si.on_wait)
                    eng = inst.engine
                    for w in waits[:-1]:
                        nop = mybir.InstNoOp(
                            name=self.nc.get_next_instruction_name(),
                            ins=[],
                            outs=[],
                            engine=eng,
                        )
                        nop.sync_info = mybir.SyncInfo(on_wait=[w], on_update=[])
                        out.append(nop)
                    inst.sync_info = mybir.SyncInfo(
                        on_wait=[waits[-1]], on_update=list(si.on_update)
                    )
                out.append(inst)
            insts[:] = out
        return orig_lower(self, ordered)

    tile.TileContext._lower_ordered_insts = _lower_with_wait_split
    tile.TileContext._ant_tail_patch = True


def build_kernel():
    _patch_tail_drain()
    nc = bass.Bass("TRN2", debug=False)

    # block-major fp8 h: [NB, 128, 3*2*512] contiguous per block
    h8d = nc.dram_tensor("h8", [NB, 128, NC2 * 2 * R], FP8, kind="ExternalInput").ap()
    w8d = nc.dram_tensor("w8", [128, MC * NC2 * 2 * 128], FP8, kind="ExternalInput").ap()
    lcd = nc.dram_tensor("lcorr", [NB, 128, 4], F16, kind="ExternalInput").ap()
    bT = nc.dram_tensor("bT", [128, MC], F32, kind="ExternalInput").ap()
    vS = nc.dram_tensor("vS", [128, MC], BF16, kind="ExternalInput").ap()
    ident = nc.dram_tensor("ident", [128, 128], F16, kind="ExternalInput").ap()
    bounce = nc.dram_tensor("bounce", [NB, R], BF16, kind="Internal").ap()
    part_out = nc.dram_tensor("part", [128, MC], F32, kind="ExternalOutput").ap()
    den_out = nc.dram_tensor("den", [4, NB], F32, kind="ExternalOutput").ap()

    with tile.TileContext(nc) as tc:
        with tc.tile_pool(name="consts", bufs=1) as consts, \
             tc.tile_pool(name="h8p", bufs=7) as h8_pool, \
             tc.tile_pool(name="h0p", bufs=1) as h0_pool, \
             tc.tile_pool(name="lcp", bufs=3) as lc_pool, \
             tc.tile_pool(name="uTp", bufs=8) as uT_pool, \
             tc.tile_pool(name="arow", bufs=2) as arow_pool, \
             tc.tile_pool(name="scp", bufs=2) as sc_pool, \
             tc.tile_pool(name="abp", bufs=3) as ab_pool, \
             tc.tile_pool(name="ttout", bufs=4) as tt_pool, \
             tc.tile_pool(name="zps", bufs=5, space="PSUM") as z_pool, \
             tc.tile_pool(name="sps", bufs=1, space="PSUM") as s_pool, \
             tc.tile_pool(name="a4ps", bufs=1, space="PSUM") as a4_pool, \
             tc.tile_pool(name="wps", bufs=1, space="PSUM") as warm_pool, \
             tc.tile_pool(name="outp", bufs=1) as out_pool:

            # --- constants loaded once (scalar-engine trigger: its own
            # DMA queue, so nothing queues behind block 0's h8). Split W8
            # in halves so block 0's first z-group waits on less. ---
            w8_sb = consts.tile([128, MC, NC2, 2, 128], FP8)
            half = MC // 2 * NC2 * 2 * 128
            nc.scalar.dma_start(
                w8_sb[:].rearrange("p jc c j m -> p (jc c j m)")[:, 0:half],
                w8d[:, 0:half],
            )
            nc.scalar.dma_start(
                w8_sb[:].rearrange("p jc c j m -> p (jc c j m)")[:, half:2 * half],
                w8d[:, half:2 * half],
            )
            bT_sb = consts.tile([128, MC], F32)
            nc.scalar.dma_start(bT_sb[:], bT)
            vS_sb = consts.tile([128, MC], BF16)
            nc.scalar.dma_start(vS_sb[:], vS)
            id_sb = consts.tile([128, 128], F16)
            nc.scalar.dma_start(id_sb[:], ident)
            shift_sb = consts.tile([4, 1], F32)
            nc.vector.memset(shift_sb[:], EXP_SHIFT)

            den_sb = out_pool.tile([4, NB], F32)
            pcol_sb = out_pool.tile([128, MC * NB], F32)  # [p, mc*NB + blk]
            part_sb = out_pool.tile([128, MC], F32)

            # Warm-up while the first DMAs are in flight: ~8.5us of dummy
            # matmuls flips the PE HAM clock gate to 2.4 GHz before real work
            # arrives, and a tiny tanh+exp pulls the ACT table loads off the
            # critical path. All operate on a zeroed scratch tile.
            warm_sb = consts.tile([128, 512], BF16)
            nc.vector.memset(warm_sb[:], 0.0)
            warm_ps = warm_pool.tile([128, R], F32, tag="warm")
            NWARM = 20
            for wi in range(NWARM):
                nc.tensor.matmul(
                    warm_ps[:],
                    lhsT=warm_sb[:, 0:128],
                    rhs=warm_sb[:],
                    start=(wi == 0),
                    stop=(wi == NWARM - 1),
                    skip_group_check=True,
                )
            warm_u = uT_pool.tile([128, 32], BF16, tag="warm_u")
            nc.scalar.activation(
                warm_u[:], warm_ps[:, 0:32],
                mybir.ActivationFunctionType.Tanh, bias=0.0, scale=1.0,
            )
            nc.scalar.activation(
                warm_u[:], warm_ps[:, 0:32],
                mybir.ActivationFunctionType.Exp, bias=0.0, scale=1.0,
            )

            # per-block live state for the software pipeline
            state = {}

            def h8_rhs(st, c):
                """[128, 2, 512] DoubleRow moving slice for pair-chunk c."""
                if "h8_chunks" in st:
                    return st["h8_chunks"][c][:]
                return st["h8"][:, c, :, :]

            def h8_ws(st, c, j):
                """[128, 512] fp8 chunk for the weighted sum (mc = 2c+j)."""
                if "h8_chunks" in st:
                    return st["h8_chunks"][c][:, j, :]
                return st["h8"][:, c, j, :]

            def emit_s_batch(blk):
                """Scores for block blk: one identity-stationary 4-col matmul
                injects the host-computed linear correction l, then per ic
                column 6 u-stationary bf16 1-col matmuls (v' = 256v), one
                sequential PSUM accumulation region."""
                st = state[blk]
                s_col = s_pool.tile([128, 4], F32)
                nc.tensor.matmul(
                    s_col[:],
                    lhsT=id_sb[:],
                    rhs=st["lc"][:],
                    start=True,
                    stop=False,
                    skip_group_check=True,
                )
                # ic outer: accumulation groups must be sequential — this
                # hardware corrupts interleaved groups within one PSUM bank
                for ic in range(4):
                    for jc in range(MC):
                        nc.tensor.matmul(
                            s_col[:, ic : ic + 1],
                            lhsT=st["u"][jc][:, ic * 128 : (ic + 1) * 128],
                            rhs=vS_sb[:, jc : jc + 1],
                            start=False,
                            stop=(jc == MC - 1),
                            skip_group_check=True,
                        )
                st.pop("lc", None)
                sc16 = sc_pool.tile([128, 4], F16)
                nc.scalar.activation(
                    sc16[:], s_col[:], mybir.ActivationFunctionType.Copy,
                    bias=0.0, scale=1.0,
                )
                st["sc16"] = sc16

            def emit_transpose(blk):
                st = state[blk]
                a4_ps = a4_pool.tile([4, 128], F16)
                nc.tensor.transpose(a4_ps[:], st["sc16"][:], id_sb[:])
                st["a4_ps"] = a4_ps

            def emit_exp_bounce(blk):
                """exp(s/256 - 10) + accum denominators, then DRAM bounce."""
                st = state[blk]
                a4row = arow_pool.tile([4, 128], BF16)
                nc.scalar.activation(
                    a4row[:], st["a4_ps"][:], mybir.ActivationFunctionType.Exp,
                    bias=shift_sb[:], scale=1.0 / SS,
                    accum_out=den_sb[:, blk : blk + 1],
                )
                nc.sync.dma_start(
                    bounce[blk : blk + 1, :].rearrange("b (c q) -> (b c) q", c=4),
                    a4row[:],
                )
                ab = ab_pool.tile([128, R], BF16)
                src = bass.AP(
                    tensor=bounce.tensor,
                    offset=bounce.offset + blk * R,
                    ap=[[0, 128], [128, 4], [1, 128]],
                )
                nc.sync.dma_start(
                    ab[:].rearrange("p (c q) -> p c q", c=4), src
                )
                st["ab"] = ab

            def emit_ws(blk):
                """fused weighted sum on fp8 h: pcol[:, mc*NB+blk] += ab*h8.
                (GpSimd can't run TensorScalarPtr — walrus ISA check — so
                all six chunks go to DVE.)"""
                st = state[blk]
                for c in range(NC2):
                    for j in range(2):
                        mc = 2 * c + j
                        eng = nc.vector
                        tt = tt_pool.tile([128, R], BF16)
                        eng.scalar_tensor_tensor(
                            out=tt[:],
                            in0=h8_ws(st, c, j),
                            scalar=1.0,
                            in1=st["ab"][:],
                            op0=mybir.AluOpType.mult,
                            op1=mybir.AluOpType.mult,
                            accum_out=pcol_sb[:, mc * NB + blk : mc * NB + blk + 1],
                        )
                state[blk] = {}

            for blk in range(NB):
                if blk == 0:
                    # block 0: one DMA per pair-chunk, alternating trigger
                    # engines, so the first z-group starts earlier
                    chunks = []
                    for c in range(NC2):
                        t = h0_pool.tile([128, 2, R], FP8, tag=f"h0{c}")
                        eng = nc.gpsimd if c % 2 == 0 else nc.sync
                        eng.dma_start(
                            t[:].rearrange("p j i -> p (j i)"),
                            h8d[0, :, c * 2 * R : (c + 1) * 2 * R],
                        )
                        chunks.append(t)
                    state[blk] = {"h8_chunks": chunks, "u": {}}
                else:
                    h8_t = h8_pool.tile([128, NC2, 2, R], FP8)
                    nc.gpsimd.dma_start(
                        h8_t[:].rearrange("p c j i -> p (c j i)"), h8d[blk]
                    )
                    state[blk] = {"h8": h8_t, "u": {}}
                lc_t = lc_pool.tile([128, 4], F16)
                nc.sync.dma_start(lc_t[:], lcd[blk])
                state[blk]["lc"] = lc_t

                for jc in range(MC):
                    z_ps = z_pool.tile([128, R], F32)
                    for c in range(NC2):
                        nc.tensor.matmul(
                            z_ps[:],
                            lhsT=w8_sb[:, jc, c, :, :],
                            rhs=h8_rhs(state[blk], c),
                            start=(c == 0),
                            stop=(c == NC2 - 1),
                            perf_mode=DR,
                        )
                    uT_t = uT_pool.tile([128, R], BF16)
                    nc.scalar.activation(
                        uT_t[:], z_ps[:], mybir.ActivationFunctionType.Tanh,
                        bias=bT_sb[:, jc : jc + 1], scale=1.0 / SW,
                    )
                    state[blk]["u"][jc] = uT_t

                    # pipelined tail work for earlier blocks, slotted between
                    # z-groups so no engine stalls on another. ws runs two
                    # blocks deep: the alpha bounce round trip gets a full
                    # block of slack.
                    if jc == 1 and blk >= 2:
                        emit_ws(blk - 2)
                    elif jc == 2 and blk >= 1:
                        emit_s_batch(blk - 1)
                    elif jc == 3 and blk >= 1:
                        emit_transpose(blk - 1)
                    elif jc == 4 and blk >= 1:
                        emit_exp_bounce(blk - 1)

            # drain the pipeline
            emit_s_batch(NB - 1)
            emit_transpose(NB - 1)
            emit_exp_bounce(NB - 1)
            emit_ws(NB - 2)
            emit_ws(NB - 1)

            # fold block partials: part[:, mc] = sum_blk pcol[:, mc*NB+blk]
            for mc in range(MC):
                nc.vector.tensor_reduce(
                    part_sb[:, mc : mc + 1],
                    pcol_sb[:, mc * NB : (mc + 1) * NB],
                    axis=mybir.AxisListType.X,
                    op=mybir.AluOpType.add,
                )

            nc.gpsimd.dma_start(part_out, part_sb[:])
            nc.gpsimd.dma_start(den_out, den_sb[:])

    return nc


def _get_engine():
    if "nc" not in _ENGINE_CACHE:
        _ENGINE_CACHE["nc"] = build_kernel()
    return _ENGINE_CACHE["nc"]


def _feat_major_to_blocks(x_fp8):
    """[H, SHARD] fp8 feature-major -> [NB, 128, 3*2*512] block-major."""
    a = np.asarray(x_fp8).reshape(NC2, 2, 128, NB, R)       # c j p blk i
    return np.ascontiguousarray(a.transpose(3, 2, 0, 1, 4).reshape(
        NB, 128, NC2 * 2 * R))


def make_in_maps(inputs):
    h_i = np.asarray(inputs["h_i"], dtype=np.float32)
    W_weight = np.asarray(inputs["W_weight"], dtype=np.float32)
    W_bias = np.asarray(inputs["W_bias"], dtype=np.float32)
    v = np.asarray(inputs["v"], dtype=np.float32)

    # fp8 h (feature-major)
    hT = np.ascontiguousarray(h_i.T)                # [H, N]
    h8 = hT.astype(NPFP8)

    # W8 stationary tiles [p, jc, c, j, m] = 16*W[jc*128+m, (2c+j)*128+p]
    W8 = (W_weight * SW).astype(NPFP8)
    W8dr = np.ascontiguousarray(
        W8.reshape(MC, 128, NC2, 2, 128).transpose(4, 0, 2, 3, 1)
        .reshape(128, MC * NC2 * 2 * 128))

    # exact linear-split correction, two host matvecs (f32 BLAS):
    #   l_i = SS * (g.h_i - g~8.h8_i)
    v16 = v.astype(NPBF16).astype(np.float32).ravel()
    g = (v16.astype(np.float64) @ W_weight.astype(np.float64)).astype(np.float32)
    g8t = ((v16.astype(np.float64) @ (W8.astype(np.float64))) / SW).astype(np.float32)
    l_full = SS * (g @ hT - g8t @ h8.astype(np.float32))     # [N] f32
    l16 = l_full.astype(np.float16)

    bT = np.ascontiguousarray(W_bias.reshape(MC, 128).T)
    vSa = np.ascontiguousarray(
        (v16 * SS).astype(NPBF16).reshape(MC, 128).T)
    identm = np.eye(128, dtype=np.float16)

    in_maps = []
    for c in range(NCORES):
        r0, r1 = c * SHARD, (c + 1) * SHARD
        lcorr = np.ascontiguousarray(
            l16[r0:r1].reshape(NB, 4, 128).transpose(0, 2, 1))
        in_maps.append({
            "h8": _feat_major_to_blocks(h8[:, r0:r1]),
            "w8": W8dr,
            "lcorr": lcorr,
            "bT": bT,
            "vS": vSa,
            "ident": identm,
        })
    return in_maps


def kernel(h_i, W_weight, W_bias, v, trace=False):
    in_maps = make_in_maps(
        {"h_i": h_i, "W_weight": W_weight, "W_bias": W_bias, "v": v}
    )
    nc = _get_engine()
    res = run_bass_kernel_spmd(
        nc, in_maps, core_ids=list(range(NCORES)), trace=trace
    )
    _ENGINE_CACHE["last_results"] = res

    num = np.zeros(H, dtype=np.float64)
    den = 0.0
    for c in range(NCORES):
        # part [128, MC]: element [p, mc] is the shard partial for
        # feature m = mc*128 + p
        part = res.results[c]["part"].astype(np.float64)
        num += part.T.reshape(H)
        den += res.results[c]["den"].astype(np.float64).sum()
    out = (num / den).astype(np.float32).reshape(1, H)
    return out


if __name__ == "__main__":
    rng = np.random.default_rng(0)
    h = rng.standard_normal((N, H), dtype=np.float32)
    W = (rng.standard_normal((H, H)) * 0.02).astype(np.float32)
    b = (rng.standard_normal(H) * 0.02).astype(np.float32)
    vv = (rng.standard_normal((1, H)) * 0.1).astype(np.float32)
    out = kernel(h, W, b, vv)
    u = np.tanh(h.astype(np.float64) @ W.astype(np.float64).T + b)
    s = (vv.astype(np.float64) @ u.T).ravel()
    a = np.exp(s - s.max())
    ref = (a @ h.astype(np.float64)) / a.sum()
    rel = np.linalg.norm(out.ravel() - ref) / np.linalg.norm(ref)
    print("rel err vs fp64 numpy ref:", rel)
